# revision 6
# baseline (speedup 1.0000x reference)
"""PolynomialAttention TRN2 kernel.

Strategy (8 NeuronCores, core = 2*b + j where b=batch, j=N-half):
  Each core computes heads 0-7 for its (batch, query-half) shard with NO
  collectives. All on-chip tensors live "transposed" (head-dim / channel on
  partitions), scores are computed as S^T[m, n] so the attention@V matmul
  needs no transpose; softmax over keys (m = partitions) uses a ones-column
  folded into the V matrix (denominator comes out of the PV matmul).
  Attention probabilities are written as [h, m, n] and untransposed on host.

Precision: q/k generation and q@k run as 3-pass bf16 hi/lo split matmuls
(~1e-5 rel err; the cubic poly amplifies score errors by ~30x so f32r's
~3e-4 is not enough). v/PV/proj/LN matmuls run in f32r. The cubic
logit polynomial is ONE custom DVE op; exp(logit - 40) on ScalarE.
"""
import sys

sys.path.insert(0, "/opt/trn_rl_repo")

import numpy as np
import ml_dtypes

import concourse.bass as bass
import concourse.bacc as bacc
import concourse.mybir as mybir
from concourse.tile import TileContext
from concourse.bass_utils import run_bass_kernel_spmd
from concourse.dve_ops import DveOp, _SUB_OPCODE_FOR_NAME, OPS, CUSTOM_DVE_SPECS
from concourse.dve_spec import Spec, Src0, C0, C1, C2, lower, _has_src1, minn
from concourse.dve_uop import DveOpSpec

B, N, C, H = 4, 1024, 512, 8
HD = C // H
SCALE = HD ** -0.5
LN_EPS = 1e-5
NH = N // 2          # per-core query rows
SHIFT = 45.0         # softmax stabilizer: exp(L - SHIFT), exact (shift-invariant)
CLAMP = 130.0        # logit clamp; rows hitting it are saturated (attn ~ 1.0)

f32 = mybir.dt.float32
f32r = mybir.dt.float32r
bf16 = mybir.dt.bfloat16
AL = mybir.AluOpType
AF = mybir.ActivationFunctionType

_np_bf16 = ml_dtypes.bfloat16


def _register_poly3():
    """out = min(((x + c0)*x + c1)*x, c2) in one DVE pass (c2 = logit clamp
    pre-divided by ow2; the ow2 factor rides the Exp activation's scale)."""
    name = "POLY3C_HORNER_ANT"
    if name in _SUB_OPCODE_FOR_NAME:
        for op in OPS:
            if op.name == name:
                return op
    spec = Spec(
        body=minn(((Src0 + C0) * Src0 + C1) * Src0, C2),
        reference=lambda in0, in1, s0, s1, imm2: np.minimum(
            ((in0.astype(np.float32) + s0) * in0 + s1) * in0, imm2
        ).astype(np.float32),
    )
    row = max(_SUB_OPCODE_FOR_NAME.values()) + 1
    _SUB_OPCODE_FOR_NAME[name] = row
    shas = {}
    for ver in ("v3", "v4"):
        s = DveOpSpec(name=name, opcode=row, uops=lower(spec, ver=ver),
                      rd1_en=_has_src1(spec))
        shas[ver] = s.sha(ver)
    op = DveOp(name, spec, subdim=False, uops_sha=shas)
    OPS.append(op)
    CUSTOM_DVE_SPECS[name] = spec
    return op


POLY3 = _register_poly3()


def _split_bf16(x):
    hi = x.astype(_np_bf16)
    lo = (x - hi.astype(np.float32)).astype(_np_bf16)
    return hi, lo


def build_nc(c1, c2, ow2):
    """Build the SPMD program. c1, c2, ow2: poly constants (baked immediates)."""
    nc = bacc.Bacc("TRN2", target_bir_lowering=False, debug=False, num_devices=8)

    d = {}
    d["xt_hi"] = nc.declare_dram_parameter("xt_hi", [C, N], bf16, isOutput=False)
    d["xt_lo"] = nc.declare_dram_parameter("xt_lo", [C, N], bf16, isOutput=False)
    d["xqt_hi"] = nc.declare_dram_parameter("xqt_hi", [C, NH], bf16, isOutput=False)
    d["xqt_lo"] = nc.declare_dram_parameter("xqt_lo", [C, NH], bf16, isOutput=False)
    d["xt_r"] = nc.declare_dram_parameter("xt_r", [C, N], f32r, isOutput=False)
    d["xqt_f"] = nc.declare_dram_parameter("xqt_f", [C, NH], f32, isOutput=False)
    d["wq_hi"] = nc.declare_dram_parameter("wq_hi", [C, C], bf16, isOutput=False)
    d["wq_lo"] = nc.declare_dram_parameter("wq_lo", [C, C], bf16, isOutput=False)
    d["wk_hi"] = nc.declare_dram_parameter("wk_hi", [C, C], bf16, isOutput=False)
    d["wk_lo"] = nc.declare_dram_parameter("wk_lo", [C, C], bf16, isOutput=False)
    d["wv_r"] = nc.declare_dram_parameter("wv_r", [C, C], f32r, isOutput=False)
    d["wp_r"] = nc.declare_dram_parameter("wp_r", [C, C], f32r, isOutput=False)
    d["bqk"] = nc.declare_dram_parameter("bqk", [2 * C], f32, isOutput=False)
    d["bv_r"] = nc.declare_dram_parameter("bv_r", [1, C], f32r, isOutput=False)
    d["bp"] = nc.declare_dram_parameter("bp", [C], f32, isOutput=False)
    d["gamma"] = nc.declare_dram_parameter("gamma", [C], f32, isOutput=False)
    d["beta"] = nc.declare_dram_parameter("beta", [C], f32, isOutput=False)
    attn_o = nc.declare_dram_parameter("attn_t", [H, N, NH], f32, isOutput=True)
    dbg_q = nc.declare_dram_parameter("dbg_q", [128, NH], f32, isOutput=True)
    dbg_k = nc.declare_dram_parameter("dbg_k", [128, N], f32, isOutput=True)
    dbg_v = nc.declare_dram_parameter("dbg_v", [128, H * (HD + 1)], f32, isOutput=True)
    dbg_z = nc.declare_dram_parameter("dbg_z", [128, 1024], f32, isOutput=True)
    dbg_e = nc.declare_dram_parameter("dbg_e", [128, 1024], f32, isOutput=True)
    dbg_d = nc.declare_dram_parameter("dbg_d", [1, 1024], f32, isOutput=True)
    dbg_rb = nc.declare_dram_parameter("dbg_rb", [128, 1024], f32, isOutput=True)
    dbg_o = nc.declare_dram_parameter("dbg_o", [64, NH], f32, isOutput=True)
    y_o = nc.declare_dram_parameter("y_t", [C, NH], f32, isOutput=True)

    with TileContext(nc) as tc, \
         tc.tile_pool(name="const", bufs=1) as const, \
         tc.tile_pool(name="qkv", bufs=1) as qkv:
        # persistent small constants
        ones_f = const.tile([128, 128], f32)
        nc.vector.memset(ones_f[:], 1.0)
        ones_r = const.tile([128, 128], f32r)
        nc.vector.tensor_copy(ones_r[:], ones_f[:])
        nbias = const.tile([128, 1], f32)
        nc.vector.memset(nbias[:], -SHIFT)
        eps_t = const.tile([1, 1], f32)
        nc.vector.memset(eps_t[:], LN_EPS)
        gam_t = const.tile([128, 4], f32)
        bet_t = const.tile([128, 4], f32)
        bp_t = const.tile([128, 4], f32)
        bqk_t = const.tile([128, 8], f32)
        for c in range(4):
            nc.sync.dma_start(gam_t[:, c:c + 1], d["gamma"][c * 128:(c + 1) * 128])
            nc.sync.dma_start(bet_t[:, c:c + 1], d["beta"][c * 128:(c + 1) * 128])
            nc.sync.dma_start(bp_t[:, c:c + 1], d["bp"][c * 128:(c + 1) * 128])
        for c in range(8):
            nc.sync.dma_start(bqk_t[:, c:c + 1], d["bqk"][c * 128:(c + 1) * 128])
        bv_t = const.tile([1, C], f32r)
        nc.sync.dma_start(bv_t[:], d["bv_r"][:])

        # persistent phase-A outputs
        qh_t = [qkv.tile([128, NH], bf16, name=f"qh{i}") for i in range(4)]
        ql_t = [qkv.tile([128, NH], bf16, name=f"ql{i}") for i in range(4)]
        kh_t = [qkv.tile([128, N], bf16, name=f"kh{i}") for i in range(4)]
        kl_t = [qkv.tile([128, N], bf16, name=f"kl{i}") for i in range(4)]
        va_t = [qkv.tile([128, H * (HD + 1)], f32r, name=f"va{i}") for i in range(8)]
        o_t = [qkv.tile([64, NH], f32r, name=f"ot{i}") for i in range(8)]
        wp8_t = [qkv.tile([64, C], f32r, name=f"wp8{i}") for i in range(8)]
        xqf_t = [qkv.tile([128, NH], f32, name=f"xqf{i}") for i in range(4)]
        for c in range(4):
            nc.sync.dma_start(xqf_t[c][:], d["xqt_f"][c * 128:(c + 1) * 128, :])
        for i in range(8):
            nc.sync.dma_start(wp8_t[i][:], d["wp_r"][i * 64:(i + 1) * 64, :])

        # ---------------- Phase A: qkv generation ----------------
        with tc.tile_pool(name="xw", bufs=1) as xw, \
             tc.tile_pool(name="psA", bufs=2, space="PSUM") as psA:
            xth = [xw.tile([128, N], bf16, name=f"xth{i}") for i in range(4)]
            xtl = [xw.tile([128, N], bf16, name=f"xtl{i}") for i in range(4)]
            xqh = [xw.tile([128, NH], bf16, name=f"xqh{i}") for i in range(4)]
            xql = [xw.tile([128, NH], bf16, name=f"xql{i}") for i in range(4)]
            xtr = [xw.tile([128, N], f32r, name=f"xtr{i}") for i in range(4)]
            wqh = [xw.tile([128, C], bf16, name=f"wqh{i}") for i in range(4)]
            wql = [xw.tile([128, C], bf16, name=f"wql{i}") for i in range(4)]
            wkh = [xw.tile([128, C], bf16, name=f"wkh{i}") for i in range(4)]
            wkl = [xw.tile([128, C], bf16, name=f"wkl{i}") for i in range(4)]
            wvr = [xw.tile([128, C], f32r, name=f"wvr{i}") for i in range(4)]
            for c in range(4):
                sl = slice(c * 128, (c + 1) * 128)
                nc.sync.dma_start(xth[c][:], d["xt_hi"][sl, :])
                nc.sync.dma_start(xtl[c][:], d["xt_lo"][sl, :])
                nc.sync.dma_start(xqh[c][:], d["xqt_hi"][sl, :])
                nc.sync.dma_start(xql[c][:], d["xqt_lo"][sl, :])
                nc.sync.dma_start(xtr[c][:], d["xt_r"][sl, :])
                nc.sync.dma_start(wqh[c][:], d["wq_hi"][sl, :])
                nc.sync.dma_start(wql[c][:], d["wq_lo"][sl, :])
                nc.sync.dma_start(wkh[c][:], d["wk_hi"][sl, :])
                nc.sync.dma_start(wkl[c][:], d["wk_lo"][sl, :])
                nc.sync.dma_start(wvr[c][:], d["wv_r"][sl, :])

            # q^T: [128, NH] per c'-chunk, 3-pass bf16 split
            for cq in range(4):
                ps = psA.tile([128, NH], f32, tag="psq")
                first = True
                for c in range(4):
                    wsl = (slice(None), slice(cq * 128, (cq + 1) * 128))
                    for wop, xop in ((wqh, xqh), (wqh, xql), (wql, xqh)):
                        nc.tensor.matmul(ps[:], wop[c][wsl], xop[c][:],
                                         start=first, stop=(c == 3 and wop is wql))
                        first = False
                bias = bqk_t[:, cq:cq + 1]
                nc.vector.tensor_scalar(qh_t[cq][:], ps[:], bias, None, AL.add)
                nc.vector.scalar_tensor_tensor(ql_t[cq][:], ps[:], bias,
                                               qh_t[cq][:], AL.add, AL.subtract)

            # k^T: [128, N] per c'-chunk
            for ck in range(4):
                ps = psA.tile([128, N], f32, tag="psk")
                for nh2 in range(2):
                    psl = slice(nh2 * 512, (nh2 + 1) * 512)
                    first = True
                    for c in range(4):
                        wsl = (slice(None), slice(ck * 128, (ck + 1) * 128))
                        xsl = (slice(None), psl)
                        for wop, xop in ((wkh, xth), (wkh, xtl), (wkl, xth)):
                            nc.tensor.matmul(ps[:, psl], wop[c][wsl], xop[c][xsl],
                                             start=first,
                                             stop=(c == 3 and wop is wkl))
                            first = False
                bias = bqk_t[:, 4 + ck:5 + ck]
                nc.vector.tensor_scalar(kh_t[ck][:], ps[:], bias, None, AL.add)
                nc.vector.scalar_tensor_tensor(kl_t[ck][:], ps[:], bias,
                                               kh_t[ck][:], AL.add, AL.subtract)

            # v natural: [128 m, C] per m-chunk (f32r) + ones cols
            for m in range(8):
                ps = psA.tile([128, C], f32, tag="psv")
                nc.tensor.matmul(ps[:], ones_r[0:1, :],
                                 bv_t[:], start=True, stop=False)
                for c in range(4):
                    nc.tensor.matmul(ps[:], xtr[c][:, m * 128:(m + 1) * 128],
                                     wvr[c][:], start=False, stop=(c == 3))
                va = va_t[m][:].rearrange("p (h w) -> p h w", w=HD + 1)
                nc.vector.tensor_copy(va[:, :, 0:HD],
                                      ps[:].rearrange("p (h w) -> p h w", w=HD))
                nc.vector.tensor_copy(
                    va[:, :, HD:HD + 1],
                    ones_f[:, 0:H].rearrange("p (h o) -> p h o", o=1))

        dqf = qkv.tile([128, NH], f32, name="dqf")
        nc.vector.tensor_tensor(dqf[:], qh_t[0][:], ql_t[0][:], AL.add)
        nc.sync.dma_start(dbg_q[:], dqf[:])
        dkf = qkv.tile([128, N], f32, name="dkf")
        nc.vector.tensor_tensor(dkf[:], kh_t[0][:], kl_t[0][:], AL.add)
        nc.sync.dma_start(dbg_k[:], dkf[:])
        nc.sync.dma_start(dbg_v[:], va_t[0][:].bitcast(f32))

        # ---------------- Phase B: attention ----------------
        with tc.tile_pool(name="zt", bufs=1) as zt, \
             tc.tile_pool(name="psS", bufs=2, space="PSUM") as psS, \
             tc.tile_pool(name="psPV", bufs=2, space="PSUM") as psPV, \
             tc.tile_pool(name="psD", bufs=1, space="PSUM") as psD:
            for hp in range(4):
                e_tiles = []
                for m in range(8):
                    s_ab = psS.tile([128, 1024], f32, tag="sab")
                    msl = slice(m * 128, (m + 1) * 128)
                    for ip, (kx, qx) in enumerate(
                            ((kh_t, qh_t), (kh_t, ql_t), (kl_t, qh_t))):
                        nc.tensor.matmul(s_ab[:, 0:512],
                                         kx[hp][0:64, msl], qx[hp][0:64, :],
                                         start=(ip == 0), stop=(ip == 2),
                                         tile_position=(0, 0))
                        nc.tensor.matmul(s_ab[:, 512:1024],
                                         kx[hp][64:128, msl], qx[hp][64:128, :],
                                         start=(ip == 0), stop=(ip == 2),
                                         tile_position=(64, 0))
                    z = zt.tile([128, 1024], f32, tag="z", bufs=3)
                    nc.vector._custom_dve(POLY3, out=z[:], in0=s_ab[:],
                                          s0=c1, s1=c2, imm2=CLAMP / ow2)
                    e = zt.tile([128, 1024], f32r, tag="e", bufs=10)
                    nc.scalar.activation(e[:], z[:], AF.Exp, bias=nbias[:, 0:1],
                                         scale=ow2)
                    e_tiles.append(e)
                    if hp == 0 and m == 0:
                        nc.sync.dma_start(dbg_z[:], z[:])
                        nc.sync.dma_start(dbg_e[:], e[:].bitcast(f32))

                d_sb = zt.tile([65, 1024], f32r, tag="dsb", bufs=2)
                pv_tiles = []
                for h2 in range(2):
                    h = 2 * hp + h2
                    pv = psPV.tile([65, 512], f32, tag="pv")
                    vsl = (slice(None), slice(h * (HD + 1), (h + 1) * (HD + 1)))
                    esl = (slice(None), slice(h2 * 512, (h2 + 1) * 512))
                    for m in range(8):
                        nc.tensor.matmul(pv[:], va_t[m][vsl], e_tiles[m][esl],
                                         start=(m == 0), stop=(m == 7))
                    nc.vector.tensor_copy(
                        d_sb[64:65, h2 * 512:(h2 + 1) * 512], pv[64:65, :])
                    pv_tiles.append(pv)

                dbc = psD.tile([128, 1024], f32, tag="dbc")
                for h2 in range(2):
                    dsl = slice(h2 * 512, (h2 + 1) * 512)
                    nc.tensor.matmul(dbc[:, dsl], ones_r[64:65, :],
                                     d_sb[64:65, dsl], start=True, stop=True)
                rb = zt.tile([128, 1024], f32, tag="rb", bufs=2)
                nc.vector.reciprocal_approx_fast(out=rb[:], in_=dbc[:])
                if hp == 0:
                    nc.sync.dma_start(dbg_d[:], d_sb[64:65, :].bitcast(f32))
                    nc.sync.dma_start(dbg_rb[:], rb[:])

                for h2 in range(2):
                    h = 2 * hp + h2
                    nc.vector.tensor_tensor(o_t[h][:], pv_tiles[h2][0:64, :],
                                            rb[0:64, h2 * 512:(h2 + 1) * 512],
                                            AL.mult)

                for m in range(8):
                    at = zt.tile([128, 1024], f32, tag="at", bufs=4)
                    nc.gpsimd.tensor_tensor(at[:], e_tiles[m][:].bitcast(f32),
                                            rb[:], AL.mult)
                    msl = slice(m * 128, (m + 1) * 128)
                    nc.sync.dma_start(attn_o[2 * hp, msl, :], at[:, 0:512])
                    nc.sync.dma_start(attn_o[2 * hp + 1, msl, :], at[:, 512:1024])

        nc.sync.dma_start(dbg_o[:], o_t[0][:].bitcast(f32))
        # ---------------- Phase C: proj + residual + LN ----------------
        with tc.tile_pool(name="cc", bufs=1) as cc, \
             tc.tile_pool(name="psC", bufs=2, space="PSUM") as psC:
            y_t = [cc.tile([128, NH], f32r, name=f"yt{i}") for i in range(4)]
            ysq = [cc.tile([128, NH], f32r, name=f"ysq{i}") for i in range(4)]
            for c in range(4):
                yp = psC.tile([128, NH], f32, tag="yp")
                for ch in range(8):
                    nc.tensor.matmul(yp[:], wp8_t[ch][:, c * 128:(c + 1) * 128],
                                     o_t[ch][:], start=(ch == 0), stop=(ch == 7))
                nc.vector.scalar_tensor_tensor(y_t[c][:], yp[:], bp_t[:, c:c + 1],
                                               xqf_t[c][:], AL.add, AL.add)
                nc.scalar.activation(ysq[c][:], y_t[c][:].bitcast(f32), AF.Square)

            mu_ps = psC.tile([1, NH], f32, tag="st1", bufs=1)
            sq_ps = psC.tile([1, NH], f32, tag="st2", bufs=1)
            for c in range(4):
                nc.tensor.matmul(mu_ps[:], ones_r[:, 0:1], y_t[c][:],
                                 start=(c == 0), stop=(c == 3))
                nc.tensor.matmul(sq_ps[:], ones_r[:, 0:1], ysq[c][:],
                                 start=(c == 0), stop=(c == 3))
            mu_r = cc.tile([1, NH], f32r)
            nc.vector.tensor_scalar(mu_r[:], mu_ps[:], 1.0 / C, None, AL.mult)
            m2 = cc.tile([1, NH], f32)
            muf = mu_r[:].bitcast(f32)
            nc.vector.tensor_tensor(m2[:], muf, muf, AL.mult)
            var = cc.tile([1, NH], f32)
            nc.vector.scalar_tensor_tensor(var[:], sq_ps[:], 1.0 / C, m2[:],
                                           AL.mult, AL.subtract)
            sd = cc.tile([1, NH], f32)
            nc.scalar.activation(sd[:], var[:], AF.Sqrt, bias=eps_t[0:1, 0:1])
            rstd = cc.tile([1, NH], f32)
            nc.vector.reciprocal_approx_fast(out=rstd[:], in_=sd[:])
            rstd_r = cc.tile([1, NH], f32r)
            nc.vector.tensor_copy(rstd_r[:], rstd[:])

            stat_ps = psC.tile([128, 1024], f32, tag="stb", bufs=1)
            nc.tensor.matmul(stat_ps[:, 0:512], ones_r[0:1, :], mu_r[:],
                             start=True, stop=True)
            nc.tensor.matmul(stat_ps[:, 512:1024], ones_r[0:1, :], rstd_r[:],
                             start=True, stop=True)
            for c in range(4):
                t1 = cc.tile([128, NH], f32, tag="t1", bufs=2)
                nc.vector.tensor_tensor(t1[:], y_t[c][:].bitcast(f32),
                                        stat_ps[:, 0:512], AL.subtract)
                t2 = cc.tile([128, NH], f32, tag="t2", bufs=2)
                nc.vector.scalar_tensor_tensor(t2[:], t1[:], gam_t[:, c:c + 1],
                                               stat_ps[:, 512:1024],
                                               AL.mult, AL.mult)
                yo = cc.tile([128, NH], f32, tag="yo", bufs=2)
                nc.vector.tensor_scalar(yo[:], t2[:], bet_t[:, c:c + 1], None,
                                        AL.add)
                nc.sync.dma_start(y_o[c * 128:(c + 1) * 128, :], yo[:])
    nc.compile()
    return nc


_BUILD_CACHE = {}


def _get_nc(ow):
    key = np.asarray(ow, np.float64).tobytes()
    if key not in _BUILD_CACHE:
        ow2 = float(ow[2])
        c1 = float(ow[1] / ow[2])
        c2 = float(ow[0] / ow[2])
        _BUILD_CACHE[key] = build_nc(c1, c2, ow2)
    return _BUILD_CACHE[key]


def kernel(x, Wqkv, bqkv, order_weights, Wproj, bproj, gamma, beta):
    x = np.ascontiguousarray(np.asarray(x, np.float32))
    Wqkv = np.ascontiguousarray(np.asarray(Wqkv, np.float32))
    bqkv = np.asarray(bqkv, np.float32)
    Wproj = np.ascontiguousarray(np.asarray(Wproj, np.float32))
    bproj = np.asarray(bproj, np.float32)
    gamma = np.asarray(gamma, np.float32)
    beta = np.asarray(beta, np.float32)
    w64 = np.asarray(order_weights, np.float64)
    e = np.exp(w64 - w64.max())
    ow = e / e.sum()

    nc = _get_nc(ow)

    wq = np.ascontiguousarray(Wqkv[:, 0:C] * np.float32(SCALE))
    wk = np.ascontiguousarray(Wqkv[:, C:2 * C])
    wv = np.ascontiguousarray(Wqkv[:, 2 * C:3 * C])
    wq_hi, wq_lo = _split_bf16(wq)
    wk_hi, wk_lo = _split_bf16(wk)
    bqk = np.concatenate([bqkv[0:C] * np.float32(SCALE), bqkv[C:2 * C]])
    bv = np.ascontiguousarray(bqkv[2 * C:3 * C].reshape(1, C))

    in_maps = []
    for core in range(8):
        b, j = divmod(core, 2)
        xt = np.ascontiguousarray(x[b].T)              # [C, N]
        xqt = np.ascontiguousarray(xt[:, j * NH:(j + 1) * NH])
        xt_hi, xt_lo = _split_bf16(xt)
        xqt_hi, xqt_lo = _split_bf16(xqt)
        in_maps.append({
            "xt_hi": xt_hi, "xt_lo": xt_lo,
            "xqt_hi": xqt_hi, "xqt_lo": xqt_lo,
            "xt_r": xt, "xqt_f": xqt,
            "wq_hi": wq_hi, "wq_lo": wq_lo,
            "wk_hi": wk_hi, "wk_lo": wk_lo,
            "wv_r": wv, "wp_r": Wproj,
            "bqk": bqk, "bv_r": bv, "bp": bproj,
            "gamma": gamma, "beta": beta,
        })

    res = run_bass_kernel_spmd(nc, in_maps, list(range(8)))

    attn = np.empty((B, H, N, N), np.float32)
    y = np.empty((B, N, C), np.float32)
    for core in range(8):
        b, j = divmod(core, 2)
        r = res.results[core]
        # attn_t: [H, m, n_local] -> attn[b, :, j*NH:(j+1)*NH, :]
        attn[b, :, j * NH:(j + 1) * NH, :] = np.swapaxes(r["attn_t"], 1, 2)
        y[b, j * NH:(j + 1) * NH, :] = r["y_t"].T
    return y, attn


# revision 8
# speedup vs baseline: 143.9784x; 143.9784x over previous
"""PolynomialAttention TRN2 kernel.

Strategy (8 NeuronCores, core = 2*b + j where b=batch, j=N-half):
  Each core computes heads 0-7 for its (batch, query-half) shard with NO
  collectives. All on-chip tensors live "transposed" (head-dim / channel on
  partitions), scores are computed as S^T[m, n] so the attention@V matmul
  needs no transpose; softmax over keys (m = partitions) uses a ones-column
  folded into the V matrix (denominator comes out of the PV matmul).
  Attention probabilities are written as [h, m, n] and untransposed on host.

Precision: q/k generation and q@k run as 3-pass bf16 hi/lo split matmuls
(~1e-5 rel err; the cubic poly amplifies score errors by ~30x so f32r's
~3e-4 is not enough). v/PV/proj/LN matmuls run in f32r. The cubic
logit polynomial is ONE custom DVE op; exp(logit - 40) on ScalarE.
"""
import sys

sys.path.insert(0, "/opt/trn_rl_repo")

import numpy as np
import ml_dtypes

import concourse.bass as bass
import concourse.bacc as bacc
import concourse.mybir as mybir
from concourse.tile import TileContext
from concourse.bass_utils import run_bass_kernel_spmd
from concourse.dve_ops import DveOp, _SUB_OPCODE_FOR_NAME, OPS, CUSTOM_DVE_SPECS
from concourse.dve_spec import Spec, Src0, C0, C1, C2, lower, _has_src1, minn
from concourse.dve_uop import DveOpSpec

B, N, C, H = 4, 1024, 512, 8
HD = C // H
SCALE = HD ** -0.5
LN_EPS = 1e-5
NH = N // 2          # per-core query rows
SHIFT = 45.0         # softmax stabilizer: exp(L - SHIFT), exact (shift-invariant)
CLAMP = 130.0        # logit clamp; rows hitting it are saturated (attn ~ 1.0)

f32 = mybir.dt.float32
f32r = mybir.dt.float32r
bf16 = mybir.dt.bfloat16
AL = mybir.AluOpType
AF = mybir.ActivationFunctionType

_np_bf16 = ml_dtypes.bfloat16


def _register_poly3():
    """out = min(((x + c0)*x + c1)*x, c2) in one DVE pass (c2 = logit clamp
    pre-divided by ow2; the ow2 factor rides the Exp activation's scale)."""
    name = "POLY3C_HORNER_ANT"
    if name in _SUB_OPCODE_FOR_NAME:
        for op in OPS:
            if op.name == name:
                return op
    spec = Spec(
        body=minn(((Src0 + C0) * Src0 + C1) * Src0, C2),
        reference=lambda in0, in1, s0, s1, imm2: np.minimum(
            ((in0.astype(np.float32) + s0) * in0 + s1) * in0, imm2
        ).astype(np.float32),
    )
    row = max(_SUB_OPCODE_FOR_NAME.values()) + 1
    _SUB_OPCODE_FOR_NAME[name] = row
    shas = {}
    for ver in ("v3", "v4"):
        s = DveOpSpec(name=name, opcode=row, uops=lower(spec, ver=ver),
                      rd1_en=_has_src1(spec))
        shas[ver] = s.sha(ver)
    op = DveOp(name, spec, subdim=False, uops_sha=shas)
    OPS.append(op)
    CUSTOM_DVE_SPECS[name] = spec
    return op


POLY3 = _register_poly3()


def _split_bf16(x):
    hi = x.astype(_np_bf16)
    lo = (x - hi.astype(np.float32)).astype(_np_bf16)
    return hi, lo


def build_nc(c1, c2, ow2, repeat=1):
    """Build the SPMD program. c1, c2, ow2: poly constants (baked immediates)."""
    nc = bacc.Bacc("TRN2", target_bir_lowering=False, debug=False, num_devices=8)

    d = {}
    d["xt_hi"] = nc.declare_dram_parameter("xt_hi", [C, N], bf16, isOutput=False)
    d["xt_lo"] = nc.declare_dram_parameter("xt_lo", [C, N], bf16, isOutput=False)
    d["xqt_hi"] = nc.declare_dram_parameter("xqt_hi", [C, NH], bf16, isOutput=False)
    d["xqt_lo"] = nc.declare_dram_parameter("xqt_lo", [C, NH], bf16, isOutput=False)
    d["xt_r"] = nc.declare_dram_parameter("xt_r", [C, N], f32r, isOutput=False)
    d["xqt_f"] = nc.declare_dram_parameter("xqt_f", [C, NH], f32, isOutput=False)
    d["wq_hi"] = nc.declare_dram_parameter("wq_hi", [C, C], bf16, isOutput=False)
    d["wq_lo"] = nc.declare_dram_parameter("wq_lo", [C, C], bf16, isOutput=False)
    d["wk_hi"] = nc.declare_dram_parameter("wk_hi", [C, C], bf16, isOutput=False)
    d["wk_lo"] = nc.declare_dram_parameter("wk_lo", [C, C], bf16, isOutput=False)
    d["wv_r"] = nc.declare_dram_parameter("wv_r", [C, C], f32r, isOutput=False)
    d["wp_r"] = nc.declare_dram_parameter("wp_r", [C, C], f32r, isOutput=False)
    d["bqk"] = nc.declare_dram_parameter("bqk", [2 * C], f32, isOutput=False)
    d["bv_r"] = nc.declare_dram_parameter("bv_r", [1, C], f32r, isOutput=False)
    d["bp"] = nc.declare_dram_parameter("bp", [C], f32, isOutput=False)
    d["gamma"] = nc.declare_dram_parameter("gamma", [C], f32, isOutput=False)
    d["beta"] = nc.declare_dram_parameter("beta", [C], f32, isOutput=False)
    attn_o = nc.declare_dram_parameter("attn_t", [H, N, NH], f32, isOutput=True)
    y_o = nc.declare_dram_parameter("y_t", [C, NH], f32, isOutput=True)

    with TileContext(nc) as tc, \
         tc.tile_pool(name="const", bufs=1) as const, \
         tc.tile_pool(name="qkv", bufs=1) as qkv:
        # persistent small constants
        ones_f = const.tile([128, 128], f32)
        nc.vector.memset(ones_f[:], 1.0)
        ones_r = const.tile([128, 128], f32r)
        nc.vector.tensor_copy(ones_r[:], ones_f[:])
        nbias = const.tile([128, 1], f32)
        nc.vector.memset(nbias[:], -SHIFT)
        eps_t = const.tile([1, 1], f32)
        nc.vector.memset(eps_t[:], LN_EPS)
        gam_t = const.tile([128, 4], f32)
        bet_t = const.tile([128, 4], f32)
        bp_t = const.tile([128, 4], f32)
        bqk_t = const.tile([128, 8], f32)
        for c in range(4):
            nc.sync.dma_start(gam_t[:, c:c + 1], d["gamma"][c * 128:(c + 1) * 128])
            nc.sync.dma_start(bet_t[:, c:c + 1], d["beta"][c * 128:(c + 1) * 128])
            nc.sync.dma_start(bp_t[:, c:c + 1], d["bp"][c * 128:(c + 1) * 128])
        for c in range(8):
            nc.sync.dma_start(bqk_t[:, c:c + 1], d["bqk"][c * 128:(c + 1) * 128])
        bv_t = const.tile([1, C], f32r)
        nc.sync.dma_start(bv_t[:], d["bv_r"][:])

        # persistent phase-A outputs
        qh_t = [qkv.tile([128, NH], bf16, name=f"qh{i}") for i in range(4)]
        ql_t = [qkv.tile([128, NH], bf16, name=f"ql{i}") for i in range(4)]
        kh_t = [qkv.tile([128, N], bf16, name=f"kh{i}") for i in range(4)]
        kl_t = [qkv.tile([128, N], bf16, name=f"kl{i}") for i in range(4)]
        va_t = [qkv.tile([128, H * (HD + 1)], f32r, name=f"va{i}") for i in range(8)]
        o_t = [qkv.tile([64, NH], f32r, name=f"ot{i}") for i in range(8)]
        wp8_t = [qkv.tile([64, C], f32r, name=f"wp8{i}") for i in range(8)]
        xqf_t = [qkv.tile([128, NH], f32, name=f"xqf{i}") for i in range(4)]
        for c in range(4):
            nc.sync.dma_start(xqf_t[c][:], d["xqt_f"][c * 128:(c + 1) * 128, :])
        for i in range(8):
            nc.sync.dma_start(wp8_t[i][:], d["wp_r"][i * 64:(i + 1) * 64, :])

        # ---------------- Phase A: qkv generation ----------------
        for _rep in range(repeat):
            _phases(nc, tc, d, locals())
    nc.compile()
    return nc


def _phases(nc, tc, d, env):
    (const, qkv, ones_f, ones_r, nbias, eps_t, gam_t, bet_t, bp_t, bqk_t, bv_t,
     qh_t, ql_t, kh_t, kl_t, va_t, o_t, wp8_t, xqf_t, attn_o, y_o, c1, c2, ow2) = (
        env["const"], env["qkv"], env["ones_f"], env["ones_r"], env["nbias"],
        env["eps_t"], env["gam_t"], env["bet_t"], env["bp_t"], env["bqk_t"],
        env["bv_t"], env["qh_t"], env["ql_t"], env["kh_t"], env["kl_t"],
        env["va_t"], env["o_t"], env["wp8_t"], env["xqf_t"], env["attn_o"],
        env["y_o"], env["c1"], env["c2"], env["ow2"])
    if True:
        with tc.tile_pool(name="xw", bufs=1) as xw, \
             tc.tile_pool(name="psA", bufs=2, space="PSUM") as psA:
            xth = [xw.tile([128, N], bf16, name=f"xth{i}") for i in range(4)]
            xtl = [xw.tile([128, N], bf16, name=f"xtl{i}") for i in range(4)]
            xqh = [xw.tile([128, NH], bf16, name=f"xqh{i}") for i in range(4)]
            xql = [xw.tile([128, NH], bf16, name=f"xql{i}") for i in range(4)]
            xtr = [xw.tile([128, N], f32r, name=f"xtr{i}") for i in range(4)]
            wqh = [xw.tile([128, C], bf16, name=f"wqh{i}") for i in range(4)]
            wql = [xw.tile([128, C], bf16, name=f"wql{i}") for i in range(4)]
            wkh = [xw.tile([128, C], bf16, name=f"wkh{i}") for i in range(4)]
            wkl = [xw.tile([128, C], bf16, name=f"wkl{i}") for i in range(4)]
            wvr = [xw.tile([128, C], f32r, name=f"wvr{i}") for i in range(4)]
            for c in range(4):
                sl = slice(c * 128, (c + 1) * 128)
                nc.sync.dma_start(xth[c][:], d["xt_hi"][sl, :])
                nc.sync.dma_start(xtl[c][:], d["xt_lo"][sl, :])
                nc.sync.dma_start(xqh[c][:], d["xqt_hi"][sl, :])
                nc.sync.dma_start(xql[c][:], d["xqt_lo"][sl, :])
                nc.sync.dma_start(xtr[c][:], d["xt_r"][sl, :])
                nc.sync.dma_start(wqh[c][:], d["wq_hi"][sl, :])
                nc.sync.dma_start(wql[c][:], d["wq_lo"][sl, :])
                nc.sync.dma_start(wkh[c][:], d["wk_hi"][sl, :])
                nc.sync.dma_start(wkl[c][:], d["wk_lo"][sl, :])
                nc.sync.dma_start(wvr[c][:], d["wv_r"][sl, :])

            # q^T: [128, NH] per c'-chunk, 3-pass bf16 split
            for cq in range(4):
                ps = psA.tile([128, NH], f32, tag="psq")
                first = True
                for c in range(4):
                    wsl = (slice(None), slice(cq * 128, (cq + 1) * 128))
                    for wop, xop in ((wqh, xqh), (wqh, xql), (wql, xqh)):
                        nc.tensor.matmul(ps[:], wop[c][wsl], xop[c][:],
                                         start=first, stop=(c == 3 and wop is wql))
                        first = False
                bias = bqk_t[:, cq:cq + 1]
                nc.vector.tensor_scalar(qh_t[cq][:], ps[:], bias, None, AL.add)
                nc.vector.scalar_tensor_tensor(ql_t[cq][:], ps[:], bias,
                                               qh_t[cq][:], AL.add, AL.subtract)

            # k^T: [128, N] per c'-chunk
            for ck in range(4):
                ps = psA.tile([128, N], f32, tag="psk")
                for nh2 in range(2):
                    psl = slice(nh2 * 512, (nh2 + 1) * 512)
                    first = True
                    for c in range(4):
                        wsl = (slice(None), slice(ck * 128, (ck + 1) * 128))
                        xsl = (slice(None), psl)
                        for wop, xop in ((wkh, xth), (wkh, xtl), (wkl, xth)):
                            nc.tensor.matmul(ps[:, psl], wop[c][wsl], xop[c][xsl],
                                             start=first,
                                             stop=(c == 3 and wop is wkl))
                            first = False
                bias = bqk_t[:, 4 + ck:5 + ck]
                nc.vector.tensor_scalar(kh_t[ck][:], ps[:], bias, None, AL.add)
                nc.vector.scalar_tensor_tensor(kl_t[ck][:], ps[:], bias,
                                               kh_t[ck][:], AL.add, AL.subtract)

            # v natural: [128 m, C] per m-chunk (f32r) + ones cols
            for m in range(8):
                ps = psA.tile([128, C], f32, tag="psv")
                nc.tensor.matmul(ps[:], ones_r[0:1, :],
                                 bv_t[:], start=True, stop=False)
                for c in range(4):
                    nc.tensor.matmul(ps[:], xtr[c][:, m * 128:(m + 1) * 128],
                                     wvr[c][:], start=False, stop=(c == 3))
                va = va_t[m][:].rearrange("p (h w) -> p h w", w=HD + 1)
                nc.vector.tensor_copy(va[:, :, 0:HD],
                                      ps[:].rearrange("p (h w) -> p h w", w=HD))
                nc.vector.tensor_copy(
                    va[:, :, HD:HD + 1],
                    ones_f[:, 0:H].rearrange("p (h o) -> p h o", o=1))

        # ---------------- Phase B: attention ----------------
        with tc.tile_pool(name="zt", bufs=1) as zt, \
             tc.tile_pool(name="psS", bufs=2, space="PSUM") as psS, \
             tc.tile_pool(name="psPV", bufs=2, space="PSUM") as psPV, \
             tc.tile_pool(name="psD", bufs=1, space="PSUM") as psD:
            for hp in range(4):
                e_tiles = []
                for m in range(8):
                    s_ab = psS.tile([128, 1024], f32, tag="sab")
                    msl = slice(m * 128, (m + 1) * 128)
                    for ip, (kx, qx) in enumerate(
                            ((kh_t, qh_t), (kh_t, ql_t), (kl_t, qh_t))):
                        nc.tensor.matmul(s_ab[:, 0:512],
                                         kx[hp][0:64, msl], qx[hp][0:64, :],
                                         start=(ip == 0), stop=(ip == 2),
                                         tile_position=(0, 0))
                        nc.tensor.matmul(s_ab[:, 512:1024],
                                         kx[hp][64:128, msl], qx[hp][64:128, :],
                                         start=(ip == 0), stop=(ip == 2),
                                         tile_position=(64, 0))
                    z = zt.tile([128, 1024], f32, tag="z", bufs=3)
                    nc.vector._custom_dve(POLY3, out=z[:], in0=s_ab[:],
                                          s0=c1, s1=c2, imm2=CLAMP / ow2)
                    e = zt.tile([128, 1024], f32r, tag="e", bufs=10)
                    nc.scalar.activation(e[:], z[:], AF.Exp, bias=nbias[:, 0:1],
                                         scale=ow2)
                    e_tiles.append(e)

                d_sb = zt.tile([65, 1024], f32r, tag="dsb", bufs=2)
                pv_tiles = []
                for h2 in range(2):
                    h = 2 * hp + h2
                    pv = psPV.tile([65, 512], f32, tag="pv")
                    vsl = (slice(None), slice(h * (HD + 1), (h + 1) * (HD + 1)))
                    esl = (slice(None), slice(h2 * 512, (h2 + 1) * 512))
                    for m in range(8):
                        nc.tensor.matmul(pv[:], va_t[m][vsl], e_tiles[m][esl],
                                         start=(m == 0), stop=(m == 7))
                    nc.vector.tensor_copy(
                        d_sb[64:65, h2 * 512:(h2 + 1) * 512], pv[64:65, :])
                    pv_tiles.append(pv)

                dbc = psD.tile([128, 1024], f32, tag="dbc")
                for h2 in range(2):
                    dsl = slice(h2 * 512, (h2 + 1) * 512)
                    nc.tensor.matmul(dbc[:, dsl], ones_r[64:65, :],
                                     d_sb[64:65, dsl], start=True, stop=True)
                rb = zt.tile([128, 1024], f32, tag="rb", bufs=2)
                nc.vector.reciprocal_approx_fast(out=rb[:], in_=dbc[:])

                for h2 in range(2):
                    h = 2 * hp + h2
                    nc.vector.tensor_tensor(o_t[h][:], pv_tiles[h2][0:64, :],
                                            rb[0:64, h2 * 512:(h2 + 1) * 512],
                                            AL.mult)

                for m in range(8):
                    at = zt.tile([128, 1024], f32, tag="at", bufs=4)
                    nc.gpsimd.tensor_tensor(at[:], e_tiles[m][:].bitcast(f32),
                                            rb[:], AL.mult)
                    msl = slice(m * 128, (m + 1) * 128)
                    nc.sync.dma_start(attn_o[2 * hp, msl, :], at[:, 0:512])
                    nc.sync.dma_start(attn_o[2 * hp + 1, msl, :], at[:, 512:1024])

        # ---------------- Phase C: proj + residual + LN ----------------
        with tc.tile_pool(name="cc", bufs=1) as cc, \
             tc.tile_pool(name="psC", bufs=2, space="PSUM") as psC:
            y_t = [cc.tile([128, NH], f32r, name=f"yt{i}") for i in range(4)]
            ysq = [cc.tile([128, NH], f32r, name=f"ysq{i}") for i in range(4)]
            for c in range(4):
                yp = psC.tile([128, NH], f32, tag="yp")
                for ch in range(8):
                    nc.tensor.matmul(yp[:], wp8_t[ch][:, c * 128:(c + 1) * 128],
                                     o_t[ch][:], start=(ch == 0), stop=(ch == 7))
                nc.vector.scalar_tensor_tensor(y_t[c][:], yp[:], bp_t[:, c:c + 1],
                                               xqf_t[c][:], AL.add, AL.add)
                nc.scalar.activation(ysq[c][:], y_t[c][:].bitcast(f32), AF.Square)

            mu_ps = psC.tile([1, NH], f32, tag="st1", bufs=1)
            sq_ps = psC.tile([1, NH], f32, tag="st2", bufs=1)
            for c in range(4):
                nc.tensor.matmul(mu_ps[:], ones_r[:, 0:1], y_t[c][:],
                                 start=(c == 0), stop=(c == 3))
                nc.tensor.matmul(sq_ps[:], ones_r[:, 0:1], ysq[c][:],
                                 start=(c == 0), stop=(c == 3))
            mu_r = cc.tile([1, NH], f32r)
            nc.vector.tensor_scalar(mu_r[:], mu_ps[:], 1.0 / C, None, AL.mult)
            m2 = cc.tile([1, NH], f32)
            muf = mu_r[:].bitcast(f32)
            nc.vector.tensor_tensor(m2[:], muf, muf, AL.mult)
            var = cc.tile([1, NH], f32)
            nc.vector.scalar_tensor_tensor(var[:], sq_ps[:], 1.0 / C, m2[:],
                                           AL.mult, AL.subtract)
            sd = cc.tile([1, NH], f32)
            nc.scalar.activation(sd[:], var[:], AF.Sqrt, bias=eps_t[0:1, 0:1])
            rstd = cc.tile([1, NH], f32)
            nc.vector.reciprocal_approx_fast(out=rstd[:], in_=sd[:])
            rstd_r = cc.tile([1, NH], f32r)
            nc.vector.tensor_copy(rstd_r[:], rstd[:])

            stat_ps = psC.tile([128, 1024], f32, tag="stb", bufs=1)
            nc.tensor.matmul(stat_ps[:, 0:512], ones_r[0:1, :], mu_r[:],
                             start=True, stop=True)
            nc.tensor.matmul(stat_ps[:, 512:1024], ones_r[0:1, :], rstd_r[:],
                             start=True, stop=True)
            for c in range(4):
                t1 = cc.tile([128, NH], f32, tag="t1", bufs=2)
                nc.vector.tensor_tensor(t1[:], y_t[c][:].bitcast(f32),
                                        stat_ps[:, 0:512], AL.subtract)
                t2 = cc.tile([128, NH], f32, tag="t2", bufs=2)
                nc.vector.scalar_tensor_tensor(t2[:], t1[:], gam_t[:, c:c + 1],
                                               stat_ps[:, 512:1024],
                                               AL.mult, AL.mult)
                yo = cc.tile([128, NH], f32, tag="yo", bufs=2)
                nc.vector.tensor_scalar(yo[:], t2[:], bet_t[:, c:c + 1], None,
                                        AL.add)
                nc.sync.dma_start(y_o[c * 128:(c + 1) * 128, :], yo[:])


_BUILD_CACHE = {}


def _get_nc(ow, repeat=1):
    key = (np.asarray(ow, np.float64).tobytes(), repeat)
    if key not in _BUILD_CACHE:
        ow2 = float(ow[2])
        c1 = float(ow[1] / ow[2])
        c2 = float(ow[0] / ow[2])
        _BUILD_CACHE[key] = build_nc(c1, c2, ow2, repeat)
    return _BUILD_CACHE[key]


def kernel(x, Wqkv, bqkv, order_weights, Wproj, bproj, gamma, beta):
    x = np.ascontiguousarray(np.asarray(x, np.float32))
    Wqkv = np.ascontiguousarray(np.asarray(Wqkv, np.float32))
    bqkv = np.asarray(bqkv, np.float32)
    Wproj = np.ascontiguousarray(np.asarray(Wproj, np.float32))
    bproj = np.asarray(bproj, np.float32)
    gamma = np.asarray(gamma, np.float32)
    beta = np.asarray(beta, np.float32)
    w64 = np.asarray(order_weights, np.float64)
    e = np.exp(w64 - w64.max())
    ow = e / e.sum()

    nc = _get_nc(ow)

    wq = np.ascontiguousarray(Wqkv[:, 0:C] * np.float32(SCALE))
    wk = np.ascontiguousarray(Wqkv[:, C:2 * C])
    wv = np.ascontiguousarray(Wqkv[:, 2 * C:3 * C])
    wq_hi, wq_lo = _split_bf16(wq)
    wk_hi, wk_lo = _split_bf16(wk)
    bqk = np.concatenate([bqkv[0:C] * np.float32(SCALE), bqkv[C:2 * C]])
    bv = np.ascontiguousarray(bqkv[2 * C:3 * C].reshape(1, C))

    in_maps = []
    for core in range(8):
        b, j = divmod(core, 2)
        xt = np.ascontiguousarray(x[b].T)              # [C, N]
        xqt = np.ascontiguousarray(xt[:, j * NH:(j + 1) * NH])
        xt_hi, xt_lo = _split_bf16(xt)
        xqt_hi, xqt_lo = _split_bf16(xqt)
        in_maps.append({
            "xt_hi": xt_hi, "xt_lo": xt_lo,
            "xqt_hi": xqt_hi, "xqt_lo": xqt_lo,
            "xt_r": xt, "xqt_f": xqt,
            "wq_hi": wq_hi, "wq_lo": wq_lo,
            "wk_hi": wk_hi, "wk_lo": wk_lo,
            "wv_r": wv, "wp_r": Wproj,
            "bqk": bqk, "bv_r": bv, "bp": bproj,
            "gamma": gamma, "beta": beta,
        })

    res = run_bass_kernel_spmd(nc, in_maps, list(range(8)))

    attn = np.empty((B, H, N, N), np.float32)
    y = np.empty((B, N, C), np.float32)
    for core in range(8):
        b, j = divmod(core, 2)
        r = res.results[core]
        # attn_t: [H, m, n_local] -> attn[b, :, j*NH:(j+1)*NH, :]
        attn[b, :, j * NH:(j + 1) * NH, :] = np.swapaxes(r["attn_t"], 1, 2)
        y[b, j * NH:(j + 1) * NH, :] = r["y_t"].T
    return y, attn
